# revision 29
# baseline (speedup 1.0000x reference)
"""MACAttention (sparse windowed attention w/ persistent memory) on 8 TRN2 cores.

Strategy: pure data parallelism over the 16 independent (batch, window)
attention blocks -- 2 windows per core, no collectives. All matmul data
is bf16 (psum accumulation stays fp32): same PE rate as float32r but
half the DMA traffic, FWL fast weight loads, and no min-256 moving-dim
constraint on masked attention chunks. Each weight tile is loaded ONCE
per core and applied to both windows (the old kernel streamed all
weights twice). Per core:
  RMSNorm (sumsq via ones-matmul; r folded into rope tables / v scale)
  QKV projection (weights streamed once, transposed layout)
  RoPE (even/odd head-dim permutation folded into weights host-side)
  windowed attention with 16 persistent k/v tokens (k on partitions,
  softmax without max subtraction, denominators via ones-matmuls,
  reciprocal_approx_fast + batched DRAM-bounce partition broadcast)
  output projection.
"""
import sys

if "/opt/trn_rl_repo" not in sys.path:
    sys.path.insert(0, "/opt/trn_rl_repo")

import numpy as np
import ml_dtypes
import concourse.bass as bass
from concourse import bacc
import concourse.mybir as mybir
import concourse.tile as tile
from concourse.bass_utils import run_bass_kernel_spmd

F32 = mybir.dt.float32
BF16 = mybir.dt.bfloat16
AF = mybir.ActivationFunctionType
BF = ml_dtypes.bfloat16

HEADS = 16
DH = 128
D = 2048
C = 512          # window width (q len)
NP = 16          # persistent tokens
NCORES = 8
NW = 2           # windows per core
T = NW * C       # tokens per core
DC = 16          # d-chunks (2048/128)
NG = 2           # head groups of 8 for softmax-denominator broadcast
SCALE = DH ** -0.5
EPS = 1e-6
THETA = 10000.0

_PERM = np.concatenate([np.arange(0, DH, 2), np.arange(1, DH, 2)])  # evens|odds


def _build():
    nc = bacc.Bacc("TRN2", target_bir_lowering=False, debug=False)

    xT = nc.declare_dram_parameter("xT", [128, DC, T], BF16, isOutput=False)
    wqk = nc.declare_dram_parameter("wqk", [32, 128, DC, 128], BF16, isOutput=False)
    wv = nc.declare_dram_parameter("wv", [4, DC, 128, C], BF16, isOutput=False)
    wo = nc.declare_dram_parameter("wo", [16, 128, DC, 128], BF16, isOutput=False)
    cos_d = nc.declare_dram_parameter("cos_d", [128, T], F32, isOutput=False)
    sin_d = nc.declare_dram_parameter("sin_d", [128, T], F32, isOutput=False)
    tri_d = nc.declare_dram_parameter("tri_d", [128, 2, 128], BF16, isOutput=False)
    pmk_d = nc.declare_dram_parameter("pmk_d", [128, HEADS, NP], BF16, isOutput=False)
    pmv_d = nc.declare_dram_parameter("pmv_d", [128, HEADS, DH], BF16, isOutput=False)
    ones_d = nc.declare_dram_parameter("ones_d", [128, 1], BF16, isOutput=False)
    out = nc.declare_dram_parameter("out", [16, 128, T], BF16, isOutput=True)
    scratch_r = nc.dram_tensor("scratch_r", [1, T], F32)
    scratch_den = nc.dram_tensor("scratch_den", [NW, 4, 4, C], BF16)

    with tile.TileContext(nc) as tc:
        with (
            tc.tile_pool(name="stat", bufs=1) as stat,
            tc.tile_pool(name="xp", bufs=1) as xp,
            tc.tile_pool(name="wvp", bufs=32) as wvp,
            tc.tile_pool(name="wp", bufs=3) as wp,
            tc.tile_pool(name="qkp", bufs=8) as qkp,
            tc.tile_pool(name="vp", bufs=8) as vp,
            tc.tile_pool(name="aop", bufs=16) as aop,
            tc.tile_pool(name="unp", bufs=8) as unp,
            tc.tile_pool(name="u0p", bufs=2) as u0p,
            tc.tile_pool(name="tmpb", bufs=2) as tmpb,
            tc.tile_pool(name="tmpf", bufs=2) as tmpf,
            tc.tile_pool(name="otp", bufs=3) as otp,
            tc.tile_pool(name="rbp", bufs=2) as rbp,
            tc.tile_pool(name="srp", bufs=2) as srp,
            tc.tile_pool(name="tabp", bufs=1) as tabp,
            tc.tile_pool(name="smallp", bufs=1) as smallp,
            tc.tile_pool(name="ps", bufs=1, space="PSUM") as ps,
        ):
            # ---- static tiles ----
            tri = stat.tile([128, 2, 128], BF16)
            nc.sync.dma_start(tri, tri_d[:, :, :])
            pmk = stat.tile([128, HEADS, NP], BF16)
            nc.sync.dma_start(pmk, pmk_d[:, :, :])
            pmv = stat.tile([128, HEADS, DH], BF16)
            nc.sync.dma_start(pmv, pmv_d[:, :, :])
            ones = stat.tile([128, 1], BF16)
            nc.sync.dma_start(ones, ones_d[:, :])
            zeros = stat.tile([128, C], BF16)
            nc.vector.memset(zeros, 0.0)
            zb = stat.tile([128, 1], F32)
            nc.vector.memset(zb, 0.0)
            epst = stat.tile([1, 1], F32)
            nc.vector.memset(epst, EPS)

            # ---- load x^T (both windows); first chunk small so the rms
            # matmuls (and the HAM warm-up window) start early ----
            xt = xp.tile([128, DC, T], BF16, tag="xt")
            for lo, hi in ((0, 1), (1, 4), (4, 8), (8, 12), (12, 16)):
                nc.sync.dma_start(xt[:, lo:hi, :], xT[:, lo:hi, :])

            # wv stream rides the scalar-engine DMA ring: issue overhead is
            # off the sync ring (which carries xt now / wqk later), and the
            # first ovb's tiles are requested before any compute
            wvt_all = []
            for ovb in range(4):
                wvt_all.append(
                    [wvp.tile([128, C], BF16, tag="wv", name=f"wv{ovb}_{dc}") for dc in range(DC)]
                )
            for dc in range(DC):
                nc.sync.dma_start(wvt_all[0][dc], wv[0, dc, :, :])

            # ---- sumsq -> r (rms scale per token), per window ----
            r_sb = stat.tile([1, T], F32)
            for w in range(NW):
                ws = slice(w * C, (w + 1) * C)
                ps_sum = ps.tile([1, C], F32, tag="sum", bufs=1, name=f"pssum{w}")
                for dc in range(DC):
                    x2 = tmpb.tile([128, C], BF16, tag="tmpb", name=f"x2_{w}_{dc}")
                    nc.vector.tensor_mul(x2, xt[:, dc, ws], xt[:, dc, ws])
                    nc.tensor.matmul(ps_sum, ones, x2, start=(dc == 0), stop=(dc == DC - 1))
                sq = smallp.tile([1, C], F32, tag="sq", name=f"sq{w}")
                nc.scalar.activation(sq, ps_sum, AF.Sqrt, bias=epst, scale=1.0 / D)
                nc.vector.reciprocal_approx_fast(out=r_sb[:, ws], in_=sq)
            # r in token-partition layout (for v scaling), via DRAM bounce.
            # Scalar-queue DMA: a sync-queue DMA here would stall the wv
            # weight stream behind the wait for r.
            nc.scalar.dma_start(scratch_r[:, :], r_sb[:, :])
            r_tp = stat.tile([128, NW * 4], F32)
            with nc.allow_non_contiguous_dma(reason="tiny r transpose"):
                nc.scalar.dma_start(r_tp, scratch_r[0].rearrange("(c p) -> p c", p=128))

            # ---- v pass: v^T tiles [128 tok, 2048 ov] per 128-token chunk ----
            v_tiles = []
            for tch in range(NW * 4):
                v_tiles.append(vp.tile([128, D], BF16, tag="v", name=f"v{tch}"))
            for ovb in range(4):
                wvt = wvt_all[ovb]
                if ovb > 0:
                    for dc in range(DC):
                        nc.scalar.dma_start(wvt[dc], wv[ovb, dc, :, :])
                for tch in range(NW * 4):
                    psv = ps.tile([128, C], F32, tag="mm", bufs=3, name=f"psv{ovb}_{tch}")
                    for dc in range(DC):
                        nc.tensor.matmul(
                            psv,
                            xt[:, dc, tch * 128 : (tch + 1) * 128],
                            wvt[dc],
                            start=(dc == 0),
                            stop=(dc == DC - 1),
                        )
                    nc.vector.tensor_scalar_mul(
                        v_tiles[tch][:, ovb * C : (ovb + 1) * C],
                        psv,
                        r_tp[:, tch : tch + 1],
                    )

            # r broadcast across partitions; fold into rope tables (tables
            # loaded after the wv stream so they don't delay the v pass)
            rbc = tabp.tile([128, T], F32, tag="rbc")
            nc.gpsimd.partition_broadcast(rbc, r_sb[:])
            cosr = tabp.tile([128, T], F32, tag="cosr")
            nc.sync.dma_start(cosr, cos_d[:, :])
            sinr = tabp.tile([128, T], F32, tag="sinr")
            nc.sync.dma_start(sinr, sin_d[:, :])
            nc.vector.tensor_mul(cosr, cosr, rbc)
            nc.vector.tensor_mul(sinr, sinr, rbc)

            # ---- qk projection + rope, software-pipelined with attention ----
            qk_tiles = [[None] * NW for _ in range(32)]
            ao_tiles = [None] * HEADS
            srow_cur = [None] * NW  # [128, C] f32; head h%4 denom on partition 32*(h%4)
            pending_norm = []  # deferred ao normalizations: (w, g, rb tile)

            def flush_norms():
                # emitted one head later than the group boundary so the rb
                # broadcast is long done: the DVE muls never block the FIFO
                for w, g, rb in pending_norm:
                    for h in range(4 * g, 4 * g + 4):
                        sl = ao_tiles[h][:, w * C : (w + 1) * C]
                        nc.vector.tensor_mul(sl, sl, rb[:, h % 4, :])
                pending_norm.clear()

            def qk_chunk(oc):
                wt = wp.tile([128, DC, 128], BF16, tag="w", name=f"wqk{oc}")
                nc.sync.dma_start(wt, wqk[oc, :, :, :])
                for w in range(NW):
                    ws = slice(w * C, (w + 1) * C)
                    pq = ps.tile([128, C], F32, tag="qk", bufs=2, name=f"pq{oc}_{w}")
                    for dc in range(DC):
                        nc.tensor.matmul(
                            pq, wt[:, dc, :], xt[:, dc, ws],
                            start=(dc == 0), stop=(dc == DC - 1),
                        )
                    # rope: qt = pq * cosr + swap_halves(pq) * sinr
                    t1 = tmpf.tile([128, C], F32, tag="tmpf", name=f"rt1_{oc}_{w}")
                    nc.vector.tensor_mul(t1[0:64], pq[64:128], sinr[0:64, ws])
                    nc.vector.tensor_mul(t1[64:128], pq[0:64], sinr[64:128, ws])
                    t2 = tmpf.tile([128, C], F32, tag="tmpf", name=f"rt2_{oc}_{w}")
                    nc.vector.tensor_mul(t2, pq, cosr[:, ws])
                    qt = qkp.tile([128, C], BF16, tag="qk", name=f"qk{oc}_{w}")
                    nc.vector.tensor_add(qt, t2, t1)
                    qk_tiles[oc][w] = qt

            def attn_head(h, w):
                flush_norms()
                q_t = qk_tiles[2 * h][w]
                k_t = qk_tiles[2 * h + 1][w]
                un = [None] * 5
                # persistent-memory chunk: sim [16, C]
                ps0 = ps.tile([16, C], F32, tag="mm", bufs=3, name=f"ps0_{h}_{w}")
                nc.tensor.matmul(ps0, pmk[:, h, :], q_t, start=True, stop=True)
                u0 = u0p.tile([128, C], BF16, tag="u0", name=f"un0_{h}_{w}")
                nc.vector.tensor_copy(u0, zeros)
                nc.scalar.activation(u0[0:16], ps0, AF.Exp, bias=zb[0:16], scale=SCALE)
                un[0] = u0
                for cch in range(1, 5):
                    cs = 128 * (cch - 1)   # diagonal block start
                    psc = ps.tile([128, C], F32, tag="mm", bufs=3, name=f"psc{h}_{w}_{cch}")
                    nc.tensor.matmul(
                        psc[:, cs:C], k_t[:, cs : cs + 128], q_t[:, cs:C],
                        start=True, stop=True,
                    )
                    uc = unp.tile([128, C], BF16, tag="un", name=f"un{h}_{w}_{cch}")
                    nc.scalar.activation(
                        uc[:, cs:C], psc[:, cs:C], AF.Exp, bias=zb, scale=SCALE
                    )
                    nc.vector.tensor_mul(
                        uc[:, cs : cs + 128],
                        uc[:, cs : cs + 128],
                        tri[:, 1 if cch > 1 else 0, :],
                    )
                    un[cch] = uc
                # denominators -> srow_cur[w] partition 32*(h%4)
                ps_s = ps.tile([1, C], F32, tag="sum", bufs=1, name=f"pss{h}_{w}")
                nc.tensor.matmul(ps_s, ones, u0, start=True, stop=False)
                for cch in range(1, 5):
                    cs = 128 * (cch - 1)
                    nc.tensor.matmul(
                        ps_s[:, cs:C], ones, un[cch][:, cs:C],
                        start=False, stop=(cch == 4),
                    )
                if h % 4 == 0:
                    srow_cur[w] = srp.tile([128, C], F32, tag="sr", name=f"srow{h}_{w}")
                j = 32 * (h % 4)
                nc.vector.tensor_copy(srow_cur[w][j : j + 1, :], ps_s)
                # attn @ v  (out^T accumulation), normalized later
                ps_av = ps.tile([128, C], F32, tag="av", bufs=2, name=f"pav{h}_{w}")
                nc.tensor.matmul(ps_av, pmv[:, h, :], u0, start=True, stop=False)
                for cch in range(1, 5):
                    cs = 128 * (cch - 1)
                    nc.tensor.matmul(
                        ps_av[:, cs:C],
                        v_tiles[w * 4 + cch - 1][:, h * DH : (h + 1) * DH],
                        un[cch][:, cs:C],
                        start=False,
                        stop=(cch == 4),
                    )
                if ao_tiles[h] is None:
                    ao_tiles[h] = aop.tile([128, T], BF16, tag="ao", name=f"ao{h}")
                nc.vector.tensor_copy(ao_tiles[h][:, w * C : (w + 1) * C], ps_av)

            def group_post(w, g):
                # heads 4g..4g+3 of window w: 1/denom, broadcast, normalize ao
                nc.vector.reciprocal_approx_fast(out=srow_cur[w], in_=srow_cur[w])
                srb = smallp.tile([128, C], BF16, tag="srb", name=f"srb{w}_{g}")
                nc.vector.tensor_copy(srb, srow_cur[w])
                for j in range(4):
                    nc.gpsimd.dma_start(
                        scratch_den[w][g][j : j + 1], srb[32 * j : 32 * j + 1, :]
                    )
                rb = rbp.tile([128, 4, C], BF16, tag="rb", name=f"rb{w}_{g}")
                sc = scratch_den[w][g]
                bsrc = bass.AP(tensor=sc.tensor, offset=sc.offset, ap=[[0, 128]] + list(sc.ap))
                nc.gpsimd.dma_start(rb, bsrc)
                pending_norm.append((w, g, rb))

            # pipeline: qk for head h, attention for head h-1
            for h in range(HEADS):
                qk_chunk(2 * h)
                qk_chunk(2 * h + 1)
                if h >= 1:
                    for w in range(NW):
                        attn_head(h - 1, w)
                    if (h - 1) % 4 == 3:
                        for w in range(NW):
                            group_post(w, (h - 1) // 4)
            for w in range(NW):
                attn_head(HEADS - 1, w)
            for w in range(NW):
                group_post(w, 3)
            flush_norms()

            # ---- output projection ----
            for ec in range(16):
                wot = wp.tile([128, DC, 128], BF16, tag="w", name=f"wo{ec}")
                nc.sync.dma_start(wot, wo[ec, :, :, :])
                ot = otp.tile([128, T], BF16, tag="ot", name=f"ot{ec}")
                for w in range(NW):
                    ws = slice(w * C, (w + 1) * C)
                    pso = ps.tile([128, C], F32, tag="mm", bufs=3, name=f"pso{ec}_{w}")
                    for hd in range(16):
                        nc.tensor.matmul(
                            pso, wot[:, hd, :], ao_tiles[hd][:, ws],
                            start=(hd == 0), stop=(hd == 15),
                        )
                    nc.vector.tensor_copy(ot[:, ws], pso)
                    nc.sync.dma_start(out[ec][:, ws], ot[:, ws])
    nc.compile()
    return nc


_NC_CACHE = None


def _get_nc():
    global _NC_CACHE
    if _NC_CACHE is None:
        _NC_CACHE = _build()
    return _NC_CACHE


def _host_prep(x, norm_w, w_qkv, w_out, pm):
    xf = np.asarray(x, np.float32)
    wq = np.asarray(w_qkv, np.float32) * np.asarray(norm_w, np.float32)[None, :]
    wof = np.asarray(w_out, np.float32)
    pmf = np.asarray(pm, np.float32)

    # wqk tiles [32, 128, 16, 128]; oc=2h -> q head h, oc=2h+1 -> k head h
    wqk_heads = wq[: 2 * D].reshape(2, HEADS, DH, D)[:, :, _PERM, :]  # [s,h,dh,d]
    wqk_t = np.empty((32, 128, DC, 128), BF)
    for h in range(HEADS):
        for s in range(2):
            blk = wqk_heads[s, h]  # [dh(o)=128, d=2048]
            wqk_t[2 * h + s] = blk.T.reshape(DC, 128, 128).transpose(1, 0, 2)

    # wv tiles [4, 16, 128, 512]: (ovb, dc, p, o) = w_v[ovb*512+o, dc*128+p]
    wv_m = wq[2 * D :]  # [2048 ov, 2048 d]
    wv_t = np.ascontiguousarray(
        wv_m.reshape(4, C, DC, 128).transpose(0, 2, 3, 1)
    ).astype(BF)

    # wo tiles [16, 128, 16, 128]: (ec, p, hdc, e) = wo[ec*128+e, hdc*128+p]
    wo_t = np.ascontiguousarray(
        wof.reshape(16, 128, 16, 128).transpose(0, 3, 2, 1)
    ).astype(BF)

    inv = THETA ** (-np.arange(0, DH, 2, dtype=np.float64) / DH)  # [64]

    # diagonal masks [128, 2, 128]: idx0 chunk-1 (longterm rows all-valid), idx1 plain
    kr = np.arange(128)[:, None]
    qq = np.arange(128)[None, :]
    tri_plain = (qq >= kr).astype(np.float32)
    tri_c1 = tri_plain.copy()
    tri_c1[0:16, :] = 1.0
    tri_t = np.ascontiguousarray(np.stack([tri_c1, tri_plain], axis=1)).astype(BF)

    pmk_t = np.ascontiguousarray(pmf[0][:, :, _PERM].transpose(2, 0, 1)).astype(BF)
    pmv_t = np.zeros((128, HEADS, DH), BF)
    pmv_t[0:16] = pmf[1].transpose(1, 0, 2)  # [16t, h, 128d] -> rows 0:16

    shared = {
        "wqk": wqk_t,
        "wv": wv_t,
        "wo": wo_t,
        "tri_d": tri_t,
        "pmk_d": pmk_t,
        "pmv_d": pmv_t,
        "ones_d": np.ones((128, 1), BF),
    }

    in_maps = []
    for c in range(NCORES):
        b, tok0 = c // 4, (c % 4) * T
        xs = xf[b, tok0 : tok0 + T]  # [1024, 2048]
        xT_c = np.ascontiguousarray(
            xs.reshape(NW, C, DC, 128).transpose(3, 2, 0, 1).reshape(128, DC, T)
        ).astype(BF)
        pos = tok0 + np.arange(T, dtype=np.float64)
        ang = pos[:, None] * inv[None, :]  # [T, 64]
        cosv = np.cos(ang).astype(np.float32).T  # [64, T]
        sinv = np.sin(ang).astype(np.float32).T
        m = dict(shared)
        m["xT"] = xT_c
        m["cos_d"] = np.ascontiguousarray(np.concatenate([cosv, cosv], axis=0))
        m["sin_d"] = np.ascontiguousarray(np.concatenate([-sinv, sinv], axis=0))
        in_maps.append(m)
    return in_maps


def kernel(x, norm_w, w_qkv, w_out, pm, _trace=False):
    nc = _get_nc()
    in_maps = _host_prep(x, norm_w, w_qkv, w_out, pm)
    res = run_bass_kernel_spmd(nc, in_maps, core_ids=list(range(NCORES)), trace=_trace)
    b, n = np.asarray(x).shape[0], np.asarray(x).shape[1]
    out_full = np.empty((b, n, D), np.float32)
    for c in range(NCORES):
        arr = np.asarray(res.results[c]["out"]).astype(np.float32)  # [16, 128, T]
        bb, tok0 = c // 4, (c % 4) * T
        out_full[bb, tok0 : tok0 + T] = arr.transpose(2, 0, 1).reshape(T, D)
    kernel._last_results = res
    return out_full


# revision 31
# speedup vs baseline: 1.0151x; 1.0151x over previous
"""MACAttention (sparse windowed attention w/ persistent memory) on 8 TRN2 cores.

Strategy: pure data parallelism over the 16 independent (batch, window)
attention blocks -- 2 windows per core, no collectives. All matmul data
is bf16 (psum accumulation stays fp32): same PE rate as float32r but
half the DMA traffic, FWL fast weight loads, and no min-256 moving-dim
constraint on masked attention chunks. Each weight tile is loaded ONCE
per core and applied to both windows (the old kernel streamed all
weights twice). Per core:
  RMSNorm (sumsq via ones-matmul; r folded into rope tables / v scale)
  QKV projection (weights streamed once, transposed layout)
  RoPE (even/odd head-dim permutation folded into weights host-side)
  windowed attention with 16 persistent k/v tokens (k on partitions,
  softmax without max subtraction, denominators via ones-matmuls,
  reciprocal_approx_fast + batched DRAM-bounce partition broadcast)
  output projection.
"""
import sys

if "/opt/trn_rl_repo" not in sys.path:
    sys.path.insert(0, "/opt/trn_rl_repo")

import numpy as np
import ml_dtypes
import concourse.bass as bass
from concourse import bacc
import concourse.mybir as mybir
import concourse.tile as tile
from concourse.bass_utils import run_bass_kernel_spmd

F32 = mybir.dt.float32
BF16 = mybir.dt.bfloat16
AF = mybir.ActivationFunctionType
BF = ml_dtypes.bfloat16

HEADS = 16
DH = 128
D = 2048
C = 512          # window width (q len)
NP = 16          # persistent tokens
NCORES = 8
NW = 2           # windows per core
T = NW * C       # tokens per core
DC = 16          # d-chunks (2048/128)
NG = 2           # head groups of 8 for softmax-denominator broadcast
SCALE = DH ** -0.5
EPS = 1e-6
THETA = 10000.0

_PERM = np.concatenate([np.arange(0, DH, 2), np.arange(1, DH, 2)])  # evens|odds


def _build():
    nc = bacc.Bacc("TRN2", target_bir_lowering=False, debug=False)

    xT = nc.declare_dram_parameter("xT", [128, DC, T], BF16, isOutput=False)
    wqk = nc.declare_dram_parameter("wqk", [32, 128, DC, 128], BF16, isOutput=False)
    wv = nc.declare_dram_parameter("wv", [4, DC, 128, C], BF16, isOutput=False)
    wo = nc.declare_dram_parameter("wo", [16, 128, DC, 128], BF16, isOutput=False)
    cos_d = nc.declare_dram_parameter("cos_d", [128, T], F32, isOutput=False)
    sin_d = nc.declare_dram_parameter("sin_d", [128, T], F32, isOutput=False)
    tri_d = nc.declare_dram_parameter("tri_d", [128, 2, 128], BF16, isOutput=False)
    pmk_d = nc.declare_dram_parameter("pmk_d", [128, HEADS, NP], BF16, isOutput=False)
    pmv_d = nc.declare_dram_parameter("pmv_d", [128, HEADS, DH], BF16, isOutput=False)
    ones_d = nc.declare_dram_parameter("ones_d", [128, 1], BF16, isOutput=False)
    out = nc.declare_dram_parameter("out", [16, 128, T], BF16, isOutput=True)
    scratch_r = nc.dram_tensor("scratch_r", [1, T], F32)
    scratch_den = nc.dram_tensor("scratch_den", [NW, 4, 4, C], BF16)

    with tile.TileContext(nc) as tc:
        with (
            tc.tile_pool(name="stat", bufs=1) as stat,
            tc.tile_pool(name="xp", bufs=1) as xp,
            tc.tile_pool(name="wvp", bufs=32) as wvp,
            tc.tile_pool(name="wp", bufs=3) as wp,
            tc.tile_pool(name="qkp", bufs=8) as qkp,
            tc.tile_pool(name="vp", bufs=8) as vp,
            tc.tile_pool(name="aop", bufs=16) as aop,
            tc.tile_pool(name="unp", bufs=8) as unp,
            tc.tile_pool(name="u0p", bufs=2) as u0p,
            tc.tile_pool(name="tmpb", bufs=2) as tmpb,
            tc.tile_pool(name="tmpf", bufs=2) as tmpf,
            tc.tile_pool(name="otp", bufs=3) as otp,
            tc.tile_pool(name="rbp", bufs=2) as rbp,
            tc.tile_pool(name="srp", bufs=2) as srp,
            tc.tile_pool(name="tabp", bufs=1) as tabp,
            tc.tile_pool(name="smallp", bufs=1) as smallp,
            tc.tile_pool(name="ps", bufs=1, space="PSUM") as ps,
        ):
            # ---- static tiles ----
            tri = stat.tile([128, 2, 128], BF16)
            nc.sync.dma_start(tri, tri_d[:, :, :])
            pmk = stat.tile([128, HEADS, NP], BF16)
            nc.sync.dma_start(pmk, pmk_d[:, :, :])
            pmv = stat.tile([128, HEADS, DH], BF16)
            nc.sync.dma_start(pmv, pmv_d[:, :, :])
            ones = stat.tile([128, 1], BF16)
            nc.sync.dma_start(ones, ones_d[:, :])
            zeros = stat.tile([128, C], BF16)
            nc.vector.memset(zeros, 0.0)
            zb = stat.tile([128, 1], F32)
            nc.vector.memset(zb, 0.0)
            epst = stat.tile([1, 1], F32)
            nc.vector.memset(epst, EPS)

            # ---- load x^T (both windows); first chunk small so the rms
            # matmuls (and the HAM warm-up window) start early ----
            xt = xp.tile([128, DC, T], BF16, tag="xt")
            for lo, hi in ((0, 1), (1, 4), (4, 8), (8, 12), (12, 16)):
                nc.sync.dma_start(xt[:, lo:hi, :], xT[:, lo:hi, :])

            # wv stream rides the scalar-engine DMA ring: issue overhead is
            # off the sync ring (which carries xt now / wqk later), and the
            # first ovb's tiles are requested before any compute
            wvt_all = []
            for ovb in range(4):
                wvt_all.append(
                    [wvp.tile([128, C], BF16, tag="wv", name=f"wv{ovb}_{dc}") for dc in range(DC)]
                )
            for dc in range(DC):
                nc.sync.dma_start(wvt_all[0][dc], wv[0, dc, :, :])

            # ---- sumsq -> r (rms scale per token), per window ----
            r_sb = stat.tile([1, T], F32)
            for w in range(NW):
                ws = slice(w * C, (w + 1) * C)
                ps_sum = ps.tile([1, C], F32, tag="sum", bufs=1, name=f"pssum{w}")
                for dc in range(DC):
                    x2 = tmpb.tile([128, C], BF16, tag="tmpb", name=f"x2_{w}_{dc}")
                    nc.vector.tensor_mul(x2, xt[:, dc, ws], xt[:, dc, ws])
                    nc.tensor.matmul(ps_sum, ones, x2, start=(dc == 0), stop=(dc == DC - 1))
                sq = smallp.tile([1, C], F32, tag="sq", name=f"sq{w}")
                nc.scalar.activation(sq, ps_sum, AF.Sqrt, bias=epst, scale=1.0 / D)
                nc.vector.reciprocal_approx_fast(out=r_sb[:, ws], in_=sq)
            # r in token-partition layout (for v scaling), via DRAM bounce.
            # Scalar-queue DMA: a sync-queue DMA here would stall the wv
            # weight stream behind the wait for r.
            nc.scalar.dma_start(scratch_r[:, :], r_sb[:, :])
            r_tp = stat.tile([128, NW * 4], F32)
            with nc.allow_non_contiguous_dma(reason="tiny r transpose"):
                nc.scalar.dma_start(r_tp, scratch_r[0].rearrange("(c p) -> p c", p=128))

            # ---- v pass: v^T tiles [128 tok, 2048 ov] per 128-token chunk ----
            v_tiles = []
            for tch in range(NW * 4):
                v_tiles.append(vp.tile([128, D], BF16, tag="v", name=f"v{tch}"))
            for ovb in range(4):
                wvt = wvt_all[ovb]
                if ovb > 0:
                    for dc in range(DC):
                        nc.scalar.dma_start(wvt[dc], wv[ovb, dc, :, :])
                for tch in range(NW * 4):
                    psv = ps.tile([128, C], F32, tag="mm", bufs=3, name=f"psv{ovb}_{tch}")
                    for dc in range(DC):
                        nc.tensor.matmul(
                            psv,
                            xt[:, dc, tch * 128 : (tch + 1) * 128],
                            wvt[dc],
                            start=(dc == 0),
                            stop=(dc == DC - 1),
                        )
                    nc.vector.tensor_scalar_mul(
                        v_tiles[tch][:, ovb * C : (ovb + 1) * C],
                        psv,
                        r_tp[:, tch : tch + 1],
                    )

            # r broadcast across partitions; fold into rope tables (tables
            # loaded after the wv stream so they don't delay the v pass)
            rbc = tabp.tile([128, T], F32, tag="rbc")
            nc.gpsimd.partition_broadcast(rbc, r_sb[:])
            cosr = tabp.tile([128, T], F32, tag="cosr")
            nc.sync.dma_start(cosr, cos_d[:, :])
            sinr = tabp.tile([128, T], F32, tag="sinr")
            nc.sync.dma_start(sinr, sin_d[:, :])
            nc.vector.tensor_mul(cosr, cosr, rbc)
            nc.vector.tensor_mul(sinr, sinr, rbc)

            # ---- qk projection + rope, software-pipelined with attention ----
            qk_tiles = [[None] * NW for _ in range(32)]
            ao_tiles = [None] * HEADS
            srow_cur = [None] * NW  # [128, C] f32; head h%4 denom on partition 32*(h%4)
            pending_norm = []  # deferred ao normalizations: (w, g, rb tile)

            def flush_norms():
                # emitted one head later than the group boundary so the rb
                # broadcast is long done: the DVE muls never block the FIFO
                for w, g, rb in pending_norm:
                    for h in range(4 * g, 4 * g + 4):
                        sl = ao_tiles[h][:, w * C : (w + 1) * C]
                        nc.vector.tensor_mul(sl, sl, rb[:, h % 4, :])
                pending_norm.clear()

            def qk_chunk(oc):
                wt = wp.tile([128, DC, 128], BF16, tag="w", name=f"wqk{oc}")
                nc.sync.dma_start(wt, wqk[oc, :, :, :])
                for w in range(NW):
                    ws = slice(w * C, (w + 1) * C)
                    pq = ps.tile([128, C], F32, tag="qk", bufs=2, name=f"pq{oc}_{w}")
                    for dc in range(DC):
                        nc.tensor.matmul(
                            pq, wt[:, dc, :], xt[:, dc, ws],
                            start=(dc == 0), stop=(dc == DC - 1),
                        )
                    # rope: qt = pq * cosr + swap_halves(pq) * sinr
                    t1 = tmpf.tile([128, C], F32, tag="tmpf", name=f"rt1_{oc}_{w}")
                    nc.vector.tensor_mul(t1[0:64], pq[64:128], sinr[0:64, ws])
                    nc.vector.tensor_mul(t1[64:128], pq[0:64], sinr[64:128, ws])
                    t2 = tmpf.tile([128, C], F32, tag="tmpf", name=f"rt2_{oc}_{w}")
                    nc.vector.tensor_mul(t2, pq, cosr[:, ws])
                    qt = qkp.tile([128, C], BF16, tag="qk", name=f"qk{oc}_{w}")
                    nc.vector.tensor_add(qt, t2, t1)
                    qk_tiles[oc][w] = qt

            def attn_head(h, w):
                flush_norms()
                q_t = qk_tiles[2 * h][w]
                k_t = qk_tiles[2 * h + 1][w]
                un = [None] * 5
                # persistent-memory chunk: sim [16, C]
                ps0 = ps.tile([16, C], F32, tag="mm", bufs=3, name=f"ps0_{h}_{w}")
                nc.tensor.matmul(ps0, pmk[:, h, :], q_t, start=True, stop=True)
                u0 = u0p.tile([128, C], BF16, tag="u0", name=f"un0_{h}_{w}")
                nc.vector.tensor_copy(u0, zeros)
                nc.scalar.activation(u0[0:16], ps0, AF.Exp, bias=zb[0:16], scale=SCALE)
                un[0] = u0
                for cch in range(1, 5):
                    cs = 128 * (cch - 1)   # diagonal block start
                    psc = ps.tile([128, C], F32, tag="mm", bufs=3, name=f"psc{h}_{w}_{cch}")
                    nc.tensor.matmul(
                        psc[:, cs:C], k_t[:, cs : cs + 128], q_t[:, cs:C],
                        start=True, stop=True,
                    )
                    uc = unp.tile([128, C], BF16, tag="un", name=f"un{h}_{w}_{cch}")
                    nc.scalar.activation(
                        uc[:, cs:C], psc[:, cs:C], AF.Exp, bias=zb, scale=SCALE
                    )
                    nc.vector.tensor_mul(
                        uc[:, cs : cs + 128],
                        uc[:, cs : cs + 128],
                        tri[:, 1 if cch > 1 else 0, :],
                    )
                    un[cch] = uc
                # denominators -> srow_cur[w] partition 32*(h%4)
                ps_s = ps.tile([1, C], F32, tag="sum", bufs=1, name=f"pss{h}_{w}")
                nc.tensor.matmul(ps_s, ones, u0, start=True, stop=False)
                for cch in range(1, 5):
                    cs = 128 * (cch - 1)
                    nc.tensor.matmul(
                        ps_s[:, cs:C], ones, un[cch][:, cs:C],
                        start=False, stop=(cch == 4),
                    )
                if h % 4 == 0:
                    srow_cur[w] = srp.tile([128, C], F32, tag="sr", name=f"srow{h}_{w}")
                j = 32 * (h % 4)
                nc.vector.tensor_copy(srow_cur[w][j : j + 1, :], ps_s)
                # attn @ v  (out^T accumulation), normalized later
                ps_av = ps.tile([128, C], F32, tag="av", bufs=2, name=f"pav{h}_{w}")
                nc.tensor.matmul(ps_av, pmv[:, h, :], u0, start=True, stop=False)
                for cch in range(1, 5):
                    cs = 128 * (cch - 1)
                    nc.tensor.matmul(
                        ps_av[:, cs:C],
                        v_tiles[w * 4 + cch - 1][:, h * DH : (h + 1) * DH],
                        un[cch][:, cs:C],
                        start=False,
                        stop=(cch == 4),
                    )
                if ao_tiles[h] is None:
                    ao_tiles[h] = aop.tile([128, T], BF16, tag="ao", name=f"ao{h}")
                nc.vector.tensor_copy(ao_tiles[h][:, w * C : (w + 1) * C], ps_av)

            def group_post(w, g):
                # heads 4g..4g+3 of window w: 1/denom, broadcast, normalize ao
                nc.vector.reciprocal_approx_fast(out=srow_cur[w], in_=srow_cur[w])
                srb = smallp.tile([128, C], BF16, tag="srb", name=f"srb{w}_{g}")
                nc.vector.tensor_copy(srb, srow_cur[w])
                for j in range(4):
                    nc.sync.dma_start(
                        scratch_den[w][g][j : j + 1], srb[32 * j : 32 * j + 1, :]
                    )
                rb = rbp.tile([128, 4, C], BF16, tag="rb", name=f"rb{w}_{g}")
                sc = scratch_den[w][g]
                bsrc = bass.AP(tensor=sc.tensor, offset=sc.offset, ap=[[0, 128]] + list(sc.ap))
                nc.sync.dma_start(rb, bsrc)
                pending_norm.append((w, g, rb))

            # pipeline: qk for head h, attention for head h-1
            for h in range(HEADS):
                qk_chunk(2 * h)
                qk_chunk(2 * h + 1)
                if h >= 1:
                    for w in range(NW):
                        attn_head(h - 1, w)
                    if (h - 1) % 4 == 3:
                        for w in range(NW):
                            group_post(w, (h - 1) // 4)
            for w in range(NW):
                attn_head(HEADS - 1, w)
            for w in range(NW):
                group_post(w, 3)
            flush_norms()

            # ---- output projection ----
            for ec in range(16):
                wot = wp.tile([128, DC, 128], BF16, tag="w", name=f"wo{ec}")
                nc.sync.dma_start(wot, wo[ec, :, :, :])
                ot = otp.tile([128, T], BF16, tag="ot", name=f"ot{ec}")
                for w in range(NW):
                    ws = slice(w * C, (w + 1) * C)
                    pso = ps.tile([128, C], F32, tag="mm", bufs=3, name=f"pso{ec}_{w}")
                    for hd in range(16):
                        nc.tensor.matmul(
                            pso, wot[:, hd, :], ao_tiles[hd][:, ws],
                            start=(hd == 0), stop=(hd == 15),
                        )
                    nc.vector.tensor_copy(ot[:, ws], pso)
                    nc.sync.dma_start(out[ec][:, ws], ot[:, ws])
    nc.compile()
    return nc


_NC_CACHE = None


def _get_nc():
    global _NC_CACHE
    if _NC_CACHE is None:
        _NC_CACHE = _build()
    return _NC_CACHE


def _host_prep(x, norm_w, w_qkv, w_out, pm):
    xf = np.asarray(x, np.float32)
    wq = np.asarray(w_qkv, np.float32) * np.asarray(norm_w, np.float32)[None, :]
    wof = np.asarray(w_out, np.float32)
    pmf = np.asarray(pm, np.float32)

    # wqk tiles [32, 128, 16, 128]; oc=2h -> q head h, oc=2h+1 -> k head h
    wqk_heads = wq[: 2 * D].reshape(2, HEADS, DH, D)[:, :, _PERM, :]  # [s,h,dh,d]
    wqk_t = np.empty((32, 128, DC, 128), BF)
    for h in range(HEADS):
        for s in range(2):
            blk = wqk_heads[s, h]  # [dh(o)=128, d=2048]
            wqk_t[2 * h + s] = blk.T.reshape(DC, 128, 128).transpose(1, 0, 2)

    # wv tiles [4, 16, 128, 512]: (ovb, dc, p, o) = w_v[ovb*512+o, dc*128+p]
    wv_m = wq[2 * D :]  # [2048 ov, 2048 d]
    wv_t = np.ascontiguousarray(
        wv_m.reshape(4, C, DC, 128).transpose(0, 2, 3, 1)
    ).astype(BF)

    # wo tiles [16, 128, 16, 128]: (ec, p, hdc, e) = wo[ec*128+e, hdc*128+p]
    wo_t = np.ascontiguousarray(
        wof.reshape(16, 128, 16, 128).transpose(0, 3, 2, 1)
    ).astype(BF)

    inv = THETA ** (-np.arange(0, DH, 2, dtype=np.float64) / DH)  # [64]

    # diagonal masks [128, 2, 128]: idx0 chunk-1 (longterm rows all-valid), idx1 plain
    kr = np.arange(128)[:, None]
    qq = np.arange(128)[None, :]
    tri_plain = (qq >= kr).astype(np.float32)
    tri_c1 = tri_plain.copy()
    tri_c1[0:16, :] = 1.0
    tri_t = np.ascontiguousarray(np.stack([tri_c1, tri_plain], axis=1)).astype(BF)

    pmk_t = np.ascontiguousarray(pmf[0][:, :, _PERM].transpose(2, 0, 1)).astype(BF)
    pmv_t = np.zeros((128, HEADS, DH), BF)
    pmv_t[0:16] = pmf[1].transpose(1, 0, 2)  # [16t, h, 128d] -> rows 0:16

    shared = {
        "wqk": wqk_t,
        "wv": wv_t,
        "wo": wo_t,
        "tri_d": tri_t,
        "pmk_d": pmk_t,
        "pmv_d": pmv_t,
        "ones_d": np.ones((128, 1), BF),
    }

    in_maps = []
    for c in range(NCORES):
        b, tok0 = c // 4, (c % 4) * T
        xs = xf[b, tok0 : tok0 + T]  # [1024, 2048]
        xT_c = np.ascontiguousarray(
            xs.reshape(NW, C, DC, 128).transpose(3, 2, 0, 1).reshape(128, DC, T)
        ).astype(BF)
        pos = tok0 + np.arange(T, dtype=np.float64)
        ang = pos[:, None] * inv[None, :]  # [T, 64]
        cosv = np.cos(ang).astype(np.float32).T  # [64, T]
        sinv = np.sin(ang).astype(np.float32).T
        m = dict(shared)
        m["xT"] = xT_c
        m["cos_d"] = np.ascontiguousarray(np.concatenate([cosv, cosv], axis=0))
        m["sin_d"] = np.ascontiguousarray(np.concatenate([-sinv, sinv], axis=0))
        in_maps.append(m)
    return in_maps


def kernel(x, norm_w, w_qkv, w_out, pm, _trace=False):
    nc = _get_nc()
    in_maps = _host_prep(x, norm_w, w_qkv, w_out, pm)
    res = run_bass_kernel_spmd(nc, in_maps, core_ids=list(range(NCORES)), trace=_trace)
    b, n = np.asarray(x).shape[0], np.asarray(x).shape[1]
    out_full = np.empty((b, n, D), np.float32)
    for c in range(NCORES):
        arr = np.asarray(res.results[c]["out"]).astype(np.float32)  # [16, 128, T]
        bb, tok0 = c // 4, (c % 4) * T
        out_full[bb, tok0 : tok0 + T] = arr.transpose(2, 0, 1).reshape(T, D)
    kernel._last_results = res
    return out_full
